# revision 10
# baseline (speedup 1.0000x reference)
"""Trainium2 Bass kernel for nn_Decoder_76836964926387.

Decoder block: upconv (ConvTranspose3d k4 s2 p1) + instance-norm + leaky,
3x3x3 correlation volume, concat, two ConvInsBlocks (3^3 conv + IN + leaky),
and a 3-channel flow head. Returns (Cn, out).

Distribution: depth-axis sharding across 8 NeuronCores (4 of 32 z-planes per
core). Instance-norm statistics via AllReduce; halo planes via AllGather with
dynamically-indexed readback; conv compute as bf16 matmuls accumulating in
PSUM (fp32). Convs run as weight-stationary interior/boundary passes so the
halo AllGathers hide under interior compute; the flow head exchanges
precomputed 27-channel boundary partial sums instead of full 219-channel
halo planes.

Self-contained: all shapes/shardings hardcoded for the fixed problem size.
"""
import sys
import os

sys.path.insert(0, '/opt/trn_rl_repo')

import numpy as np
import ml_dtypes

import concourse.bass as bass
import concourse.bacc as bacc
import concourse.tile as tile
import concourse.mybir as mybir

BF16 = ml_dtypes.bfloat16
dt = mybir.dt
AF = mybir.ActivationFunctionType
ALU = mybir.AluOpType

NCORES = 8
D = 32              # full volume depth/height/width
PD = 4              # own z-planes per core
PL = 34             # padded plane edge
PLSZ = PL * PL      # 1156
NPLANES = 6         # own 4 + 2 halo
CFULL = 219         # x / conv channels
CC0, CC1 = 128, 91  # channel chunks
NTOT = float(D * D * D)  # instance-norm element count
EPS = 1e-5
ALPHA = 0.1
CP = 18 * 18        # padded C-slab plane (16+2)^2

RG = [list(range(NCORES))]

INT_BLOCKS = [(1, 0), (1, 1), (2, 0), (2, 1)]
BND_BLOCKS = [(0, 0), (0, 1), (3, 0), (3, 1)]


def tap_idx(dz, dy, dx):
    return (dz + 1) * 9 + (dy + 1) * 3 + (dx + 1)


# correlation tap pairing: lo tap, hi = lo with dx/dy +1 instead of -1.
# 'A' pairs share the +2 pre-shifted src copy, 'B' pairs the +68 one.
PAIRS = (
    [((dz, dy, -1), 'A') for dz in (-1, 0, 1) for dy in (-1, 0, 1)]
    + [((dz, -1, 0), 'B') for dz in (-1, 0, 1)]
)
SINGLES = [(-1, 0, 0), (1, 0, 0), (0, 0, 0)]


def _pair_hi(lo, kind):
    dz, dy, dx = lo
    return (dz, dy, 1) if kind == 'A' else (dz, 1, 0)


def cost_row_map():
    """x-tile1 partition row for each corr tap (rows 64..90)."""
    rows = {}
    for j, (lo, kind) in enumerate(PAIRS):
        rows[tap_idx(*lo)] = 64 + 2 * j
        rows[tap_idx(*_pair_hi(lo, kind))] = 64 + 2 * j + 1
    for i, t in enumerate(SINGLES):
        rows[tap_idx(*t)] = 88 + i
    return rows


def off3(dz, dy, dx):
    return dz * PLSZ + dy * PL + dx


# ---------------------------------------------------------------------------
# program builder
# ---------------------------------------------------------------------------

def build_program():
    nc = bacc.Bacc("TRN2", target_bir_lowering=False, debug=False,
                   num_devices=NCORES)

    f32, bf16, u32 = dt.float32, dt.bfloat16, dt.uint32

    # ---- kernel I/O ----
    x0d = nc.dram_tensor("x0", [128, NPLANES * PLSZ], bf16, kind="ExternalInput")
    csd = [nc.dram_tensor(f"cs{k}", [128, 4 * CP], bf16, kind="ExternalInput") for k in range(4)]
    wud = [nc.dram_tensor(f"wu{k}", [128, 4096], bf16, kind="ExternalInput") for k in range(4)]
    w1d = [nc.dram_tensor(f"w1k{k}", [128, 27 * CFULL], bf16, kind="ExternalInput") for k in range(2)]
    w2d = [nc.dram_tensor(f"w2k{k}", [128, 27 * CFULL], bf16, kind="ExternalInput") for k in range(2)]
    wod = [nc.dram_tensor(f"wo{k}", [128, 81], bf16, kind="ExternalInput") for k in range(2)]
    ed = nc.dram_tensor("ec", [27, 27], bf16, kind="ExternalInput")
    redd = nc.dram_tensor("red", [128, 32], bf16, kind="ExternalInput")
    b1d = [nc.dram_tensor(f"b1c{k}", [128, 1], f32, kind="ExternalInput") for k in range(2)]
    b2d = [nc.dram_tensor(f"b2c{k}", [128, 1], f32, kind="ExternalInput") for k in range(2)]
    bud = nc.dram_tensor("bu", [64, 1], f32, kind="ExternalInput")
    bod = nc.dram_tensor("bo", [3, 1], f32, kind="ExternalInput")
    maskd = nc.dram_tensor("mask", [128, 2], f32, kind="ExternalInput")
    hidxd = nc.dram_tensor("hidx", [1, 2], u32, kind="ExternalInput")

    cn0d = nc.dram_tensor("cn0", [128, PD * PLSZ], bf16, kind="ExternalOutput")
    cn1d = nc.dram_tensor("cn1", [CC1, PD * PLSZ], bf16, kind="ExternalOutput")
    flowd = nc.dram_tensor("flow", [3, PD * 1024], f32, kind="ExternalOutput")

    # ---- collective bounce buffers ----
    ag_shapes = [(2, CC1, PLSZ), (2, CFULL, PLSZ), (2, 27, 1024)]
    ag_in = [nc.dram_tensor(f"agi{i}", list(s), bf16, kind="Internal")
             for i, s in enumerate(ag_shapes)]
    ag_out = [nc.dram_tensor(f"ago{i}", [2 * NCORES] + list(s[1:]), bf16,
                             kind="Internal", addr_space="Shared")
              for i, s in enumerate(ag_shapes)]
    ar_in = [nc.dram_tensor("ari0", [64, 2], f32, kind="Internal"),
             [nc.dram_tensor(f"ari1_{m}", [128, 2], f32, kind="Internal") for m in range(2)],
             [nc.dram_tensor(f"ari2_{m}", [128, 2], f32, kind="Internal") for m in range(2)]]
    ar_out = [nc.dram_tensor("aro0", [64, 2], f32, kind="Internal", addr_space="Shared"),
              [nc.dram_tensor(f"aro1_{m}", [128, 2], f32, kind="Internal",
                              addr_space="Shared") for m in range(2)],
              [nc.dram_tensor(f"aro2_{m}", [128, 2], f32, kind="Internal",
                              addr_space="Shared") for m in range(2)]]

    with tile.TileContext(nc) as tc:
        with tc.tile_pool(name="cp", bufs=1) as cp, \
             tc.tile_pool(name="wp", bufs=1) as wp, \
             tc.tile_pool(name="xp", bufs=1) as xp, \
             tc.tile_pool(name="big", bufs=1) as big, \
             tc.tile_pool(name="prod", bufs=1) as prod, \
             tc.tile_pool(name="rawp", bufs=1) as rawp, \
             tc.tile_pool(name="scr", bufs=1) as scr, \
             tc.tile_pool(name="stat", bufs=1) as stat, \
             tc.tile_pool(name="ps", bufs=8, space="PSUM") as ps:

            sync = nc.sync
            act = nc.scalar
            vec = nc.vector

            # ---------------- consts ----------------
            hidx_sb = cp.tile([1, 2], u32, tag="hidx")
            sync.dma_start(hidx_sb, hidxd[:, :])
            mask_sb = cp.tile([128, 2], f32, tag="mask")
            sync.dma_start(mask_sb, maskd[:, :])
            red_sb = cp.tile([128, 32], bf16, tag="red")
            sync.dma_start(red_sb, redd[:, :])
            e_sb = cp.tile([27, 27], bf16, tag="ec")
            sync.dma_start(e_sb, ed[:, :])
            wo_sb = [cp.tile([128, 3, 27], bf16, tag=f"wo{k}", name=f"wo_sb{k}") for k in range(2)]
            for k in range(2):
                sync.dma_start(wo_sb[k], wod[k][:, :])
            b1_sb = [cp.tile([128, 1], f32, tag=f"b1_{k}", name=f"b1_sb{k}") for k in range(2)]
            b2_sb = [cp.tile([128, 1], f32, tag=f"b2_{k}", name=f"b2_sb{k}") for k in range(2)]
            for k in range(2):
                sync.dma_start(b1_sb[k], b1d[k][:, :])
                sync.dma_start(b2_sb[k], b2d[k][:, :])
            bu_sb = cp.tile([64, 1], f32, tag="bu")
            sync.dma_start(bu_sb, bud[:, :])
            bo_sb = cp.tile([3, 1], f32, tag="bo")
            sync.dma_start(bo_sb, bod[:, :])
            eps_sb = cp.tile([128, 1], f32, tag="epsc")
            vec.memset(eps_sb, EPS)

            # halo indices -> registers
            r_lo = nc.alloc_registers("r_lo")
            nc.regs_load(r_lo, hidx_sb[0:1, 0:1])
            sv_lo = nc.snap(r_lo, donate=True)
            r_hi = nc.alloc_registers("r_hi")
            nc.regs_load(r_hi, hidx_sb[0:1, 1:2])
            sv_hi = nc.snap(r_hi, donate=True)

            # ---------------- input / weight loads ----------------
            cs_sb = [wp.tile([128, 4, 18, 18], bf16, tag=f"wa{k}", name=f"cs_sb{k}") for k in range(4)]
            for k in range(4):
                sync.dma_start(cs_sb[k], csd[k][:, :])
            wu_sb = [wp.tile([128, 8, 8, 64], bf16, tag=f"wb{k}", name=f"wu_sb{k}") for k in range(4)]
            for k in range(4):
                sync.dma_start(wu_sb[k], wud[k][:, :])

            x0 = xp.tile([128, NPLANES, PL, PL], bf16, tag="xslot0")
            sync.dma_start(x0, x0d[:, :])
            x1 = xp.tile([128, NPLANES, PL, PL], bf16, tag="xslot1")
            nc.gpsimd.memset(x1, 0.0)

            # c1 weights: chunk0 borrows the (late-used) cn1 big slot; chunk1
            # reuses the C-slab slot that frees after the upconv's kc0 sweep.
            w1_sb = [big.tile([128, 27, CFULL], bf16, tag="bigslot3", name="w1_sb0"),
                     wp.tile([128, 27, CFULL], bf16, tag="wa0", name="w1_sb1")]
            sync.dma_start(w1_sb[0], w1d[0][:, :])
            sync.dma_start(w1_sb[1], w1d[1][:, :])

            # corr pre-shifted src copies (emitted early so DMA queues warm)
            tgt2 = big.tile([128, NPLANES, PL, PL], bf16, tag="bigslot0")
            srcA = big.tile([128, NPLANES, PL, PL], bf16, tag="bigslot1")
            srcB = big.tile([128, NPLANES, PL, PL], bf16, tag="bigslot2")
            t2f = tgt2.rearrange("p a b c -> p (a b c)")
            sAf = srcA.rearrange("p a b c -> p (a b c)")
            sBf = srcB.rearrange("p a b c -> p (a b c)")
            x0f = x0.rearrange("p a b c -> p (a b c)")
            FS = NPLANES * PLSZ
            nc.gpsimd.memset(srcA[64:128, :, :, :], 0.0)
            nc.gpsimd.memset(srcB[64:128, :, :, :], 0.0)
            sync.dma_start(t2f[0:64, :], x0f[0:64, :])
            sync.dma_start(t2f[64:128, :], x0f[0:64, :])
            sync.dma_start(sAf[0:64, :], x0f[64:128, :])
            sync.dma_start(sAf[64:128, 0:FS - 2], x0f[64:128, 2:FS])
            sync.dma_start(sBf[0:64, :], x0f[64:128, :])
            sync.dma_start(sBf[64:128, 0:FS - 68], x0f[64:128, 68:FS])

            # ---------------- corr products + reduction ----------------
            QLO, QHI = 35, 4 * PLSZ - 35
            cost_dmas = []
            for g in range(6):
                pts = []
                for gi in range(2):
                    j = 2 * g + gi
                    lo, kind = PAIRS[j]
                    pt = prod.tile([128, PD, PL, PL], bf16, tag=f"prodslot{gi}")
                    ptf = pt.rearrange("p a b c -> p (a b c)")
                    srcf = sAf if kind == 'A' else sBf
                    d0 = off3(*lo)
                    vec.tensor_mul(ptf[:, QLO:QHI],
                                   t2f[:, PLSZ + QLO: PLSZ + QHI],
                                   srcf[:, PLSZ + QLO + d0: PLSZ + QHI + d0])
                    pts.append(pt)
                for b, (d, h) in enumerate(INT_BLOCKS + BND_BLOCKS):
                    pt_ps = ps.tile([128, 16, 32], f32, tag="psacc")
                    for gi in range(2):
                        rhs = pts[gi][:, d, 16 * h + 1: 16 * h + 17, 1:33]
                        nc.tensor.matmul(pt_ps[32 * gi: 32 * gi + 32, :, :],
                                         red_sb, rhs,
                                         tile_position=(0, 32 * gi))
                    st = scr.tile([128, 16, 32], bf16, tag=f"cstg{b % 2}")
                    act.activation(st[0:34, :, :], pt_ps[0:34, :, :], AF.Copy)
                    for gi in range(2):
                        j = 2 * g + gi
                        cost_dmas.append((st, 32 * gi, 64 + 2 * j, 2, d, h))
            for i, tp in enumerate(SINGLES):
                pt = prod.tile([128, PD, PL, PL], bf16, tag=f"prodslot{i % 2}")
                ptf = pt.rearrange("p a b c -> p (a b c)")
                d0 = off3(*tp)
                vec.tensor_mul(ptf[0:64, QLO:QHI],
                               t2f[0:64, PLSZ + QLO: PLSZ + QHI],
                               sAf[0:64, PLSZ + QLO + d0: PLSZ + QHI + d0])
                for b, (d, h) in enumerate(INT_BLOCKS + BND_BLOCKS):
                    pt_ps = ps.tile([128, 16, 32], f32, tag="psacc")
                    rhs = pt[0:64, d, 16 * h + 1: 16 * h + 17, 1:33]
                    nc.tensor.matmul(pt_ps[0:32, :, :], red_sb[0:64, 0:32], rhs)
                    st = scr.tile([128, 16, 32], bf16, tag=f"cstg{b % 2}")
                    act.activation(st[0:1, :, :], pt_ps[0:1, :, :], AF.Copy)
                    cost_dmas.append((st, 0, 88 + i, 1, d, h))
            for st, srow, drow, n, d, h in cost_dmas:
                sync.dma_start(x1[drow:drow + n, d + 1, 16 * h + 1: 16 * h + 17, 1:33],
                               st[srow:srow + n, :, :])

            # ---------------- upconv matmuls: two passes of 4 parities ------
            cup_raw = rawp.tile([64, 8, 512], bf16, tag="rawslot0")
            sums_u = stat.tile([64, 16], f32, tag="sumsu")
            for half in range(2):
                psu = [ps.tile([64, 2, 16, 16], f32, tag="psacc", name=f"psu{half}_{j}")
                       for j in range(4)]
                for k in range(4):
                    for j in range(4):
                        p = 4 * half + j
                        pz, py, px = p >> 2 & 1, p >> 1 & 1, p & 1
                        for t in range(8):
                            tz, ty, tx = t >> 2 & 1, t >> 1 & 1, t & 1
                            rhs = cs_sb[k][:, tz + pz: tz + pz + 2,
                                           ty + py: ty + py + 16,
                                           tx + px: tx + px + 16]
                            nc.tensor.matmul(psu[j], wu_sb[k][:, p, t, :], rhs,
                                             start=(k == 0 and t == 0),
                                             stop=(k == 3 and t == 7))
                for j in range(4):
                    p = 4 * half + j
                    act.activation(cup_raw[:, p, :].rearrange("p (a b c) -> p a b c", a=2, b=16, c=16),
                                   psu[j], AF.Identity, bias=bu_sb, scale=1.0,
                                   accum_out=sums_u[:, p: p + 1])
                    sq = scr.tile([128, 16, 32], bf16, tag=f"scrslot{j % 2}")
                    act.activation(sq[0:64, 0:16, 0:32].rearrange("p a b -> p (a b)"),
                                   cup_raw[:, p, :], AF.Square,
                                   accum_out=sums_u[:, 8 + p: 9 + p])

            # ---------------- AR#1: upconv instance-norm stats --------------
            st_u = stat.tile([64, 2], f32, tag="aru")
            vec.reduce_sum(st_u[:, 0:1], sums_u[:, 0:8], axis=mybir.AxisListType.X)
            vec.reduce_sum(st_u[:, 1:2], sums_u[:, 8:16], axis=mybir.AxisListType.X)
            sync.dma_start(ar_in[0][:, :], st_u)
            nc.gpsimd.collective_compute("AllReduce", ALU.add, replica_groups=RG,
                                         ins=[ar_in[0][:, :]], outs=[ar_out[0][:, :]])
            g_u = stat.tile([64, 2], f32, tag="gu")
            sync.dma_start(g_u, ar_out[0][:, :])

            def norm_coeffs(g, n, tagp):
                m = stat.tile([n, 1], f32, tag=tagp + "m", name=tagp + "m")
                vec.tensor_scalar_mul(m, g[0:n, 0:1], 1.0 / NTOT)
                q = stat.tile([n, 1], f32, tag=tagp + "q", name=tagp + "q")
                vec.tensor_scalar_mul(q, g[0:n, 1:2], 1.0 / NTOT)
                v = stat.tile([n, 1], f32, tag=tagp + "v", name=tagp + "v")
                vec.scalar_tensor_tensor(v, m, -1.0, m, ALU.mult, ALU.mult)
                vec.tensor_add(v, q, v)
                u = stat.tile([n, 1], f32, tag=tagp + "u", name=tagp + "u")
                act.activation(u, v, AF.Sqrt, bias=eps_sb[0:n, :], scale=1.0)
                s = stat.tile([n, 1], f32, tag=tagp + "s", name=tagp + "s")
                vec.reciprocal(s, u)
                bb = stat.tile([n, 1], f32, tag=tagp + "b", name=tagp + "b")
                vec.scalar_tensor_tensor(bb, m, -1.0, s, ALU.mult, ALU.mult)
                return s, bb

            s_u, b_u = norm_coeffs(g_u, 64, "u")

            # upconv norm + leaky -> x1 rows 0:64 (strided parity writes)
            for p in range(8):
                pz, py, px = p >> 2 & 1, p >> 1 & 1, p & 1
                a_t = scr.tile([128, 16, 32], bf16, tag=f"scrslot{p % 2}")
                a_v = a_t[0:64, 0:16, 0:32].rearrange("p a b -> p (a b)")
                act.activation(a_v, cup_raw[:, p, :], AF.Identity,
                               bias=b_u, scale=s_u)
                a_r = a_t[0:64, 0:16, 0:32].rearrange("p a b -> p (a b)").rearrange(
                    "p (a b c) -> p a b c", a=2, b=16, c=16)
                for iz in range(2):
                    dst = x1[0:64, 1 + pz + 2 * iz, 1 + py: 33: 2, 1 + px: 33: 2]
                    vec.scalar_tensor_tensor(dst, a_r[:, iz, :, :], ALPHA,
                                             a_r[:, iz, :, :], ALU.mult, ALU.max)

            # ---------------- AG#1: x1 halo planes ----------------
            sync.dma_start(ag_in[0][0, :, :], x1[0:CC1, 1, :, :])
            sync.dma_start(ag_in[0][1, :, :], x1[0:CC1, 4, :, :])
            nc.gpsimd.collective_compute("AllGather", ALU.bypass, replica_groups=RG,
                                         ins=[ag_in[0][:, :, :]], outs=[ag_out[0][:, :, :]])
            sync.dma_start(x1[0:CC1, 0, :, :], ag_out[0][bass.ds(sv_lo, 1), :, :])
            sync.dma_start(x1[0:CC1, 5, :, :], ag_out[0][bass.ds(sv_hi, 1), :, :])
            vec.tensor_scalar_mul(x1[0:CC1, 0, :, :], x1[0:CC1, 0, :, :], mask_sb[0:CC1, 0:1])
            vec.tensor_scalar_mul(x1[0:CC1, 5, :, :], x1[0:CC1, 5, :, :], mask_sb[0:CC1, 1:2])

            # ---------------- generic conv + IN + leaky stage ----------------
            def conv_stage(xin_tiles, w_tiles, b_tiles, ar_i, ar_o, out_tiles,
                           tagp, post03=None):
                """Weight-stationary 3^3 conv passes + IN stats + leaky.

                Four passes: (mc0,int), (mc1,int), (mc0,bnd), (mc1,bnd) — the
                boundary passes run last so the previous stage's halo exchange
                hides under interior compute. post03() fires after output
                planes 0 and 3 are normalized (to kick the next exchange).
                """
                raws = [rawp.tile([128, PD, 32, 32], bf16, tag=f"rawslot{mc}",
                                  name=f"{tagp}raw{mc}") for mc in range(2)]
                sums = [stat.tile([128, 16], f32, tag=tagp + f"sums{mc}",
                                  name=f"{tagp}sums{mc}") for mc in range(2)]
                passes = [(0, INT_BLOCKS, 0), (0, BND_BLOCKS, 4),
                          (1, INT_BLOCKS, 0), (1, BND_BLOCKS, 4)]
                coeffs = [None, None]
                for mc, blks, coff in passes:
                    mlen = CC0 if mc == 0 else CC1
                    pts = [ps.tile([128, 16, 32], f32, tag="psacc",
                                   name=f"{tagp}ps{mc}{coff}{j}") for j in range(4)]
                    for t in range(27):
                        tz, ty, tx = t // 9, (t // 3) % 3, t % 3
                        for k in range(2):
                            w_ap = w_tiles[k][:, t, 128 * mc: 128 * mc + mlen]
                            for j, (d, h) in enumerate(blks):
                                rhs = xin_tiles[k][:, d + tz,
                                                   16 * h + ty: 16 * h + ty + 16,
                                                   tx: tx + 32]
                                nc.tensor.matmul(
                                    pts[j][0:mlen, :, :], w_ap, rhs,
                                    start=(t == 0 and k == 0),
                                    stop=(t == 26 and k == 1))
                    for j, (d, h) in enumerate(blks):
                        act.activation(raws[mc][0:mlen, d, 16 * h: 16 * h + 16, :],
                                       pts[j][0:mlen, :, :], AF.Identity,
                                       bias=b_tiles[mc][0:mlen, :], scale=1.0,
                                       accum_out=sums[mc][0:mlen, coff + j: coff + j + 1])
                        sq = scr.tile([128, 16, 32], bf16, tag=f"scrslot{j % 2}")
                        act.activation(sq[0:mlen, :, :],
                                       raws[mc][0:mlen, d, 16 * h: 16 * h + 16, :],
                                       AF.Square,
                                       accum_out=sums[mc][0:mlen, 8 + coff + j: 9 + coff + j])
                    if coff == 4:
                        # this mc's stats are complete: allreduce them while the
                        # other chunk's passes still run on the PE
                        stt = stat.tile([128, 2], f32, tag=tagp + f"st{mc}",
                                        name=f"{tagp}st{mc}")
                        vec.reduce_sum(stt[0:mlen, 0:1], sums[mc][0:mlen, 0:8],
                                       axis=mybir.AxisListType.X)
                        vec.reduce_sum(stt[0:mlen, 1:2], sums[mc][0:mlen, 8:16],
                                       axis=mybir.AxisListType.X)
                        sync.dma_start(ar_i[mc][0:mlen, :], stt[0:mlen, :])
                        nc.gpsimd.collective_compute(
                            "AllReduce", ALU.add, replica_groups=RG,
                            ins=[ar_i[mc][0:mlen, :]], outs=[ar_o[mc][0:mlen, :]])
                        g = stat.tile([128, 2], f32, tag=tagp + f"g{mc}",
                                      name=f"{tagp}g{mc}")
                        sync.dma_start(g[0:mlen, :], ar_o[mc][0:mlen, :])
                        coeffs[mc] = norm_coeffs(g, mlen, tagp + f"c{mc}")
                for d in (0, 3, 1, 2):
                    for mc in range(2):
                        mlen = CC0 if mc == 0 else CC1
                        s, bb = coeffs[mc]
                        a_t = prod.tile([128, 32, 32], bf16, tag=f"prodslot{mc % 2}",
                                        name=f"{tagp}a{mc}{d}")
                        act.activation(a_t[0:mlen, :, :], raws[mc][0:mlen, d, :, :],
                                       AF.Identity, bias=bb, scale=s)
                        vec.scalar_tensor_tensor(
                            out_tiles[mc][0:mlen, d + 1, 1:33, 1:33],
                            a_t[0:mlen, :, :], ALPHA, a_t[0:mlen, :, :],
                            ALU.mult, ALU.max)
                    if d == 3 and post03 is not None:
                        post03()

            def halo_exchange(tiles, agi, ago):
                sync.dma_start(agi[0, 0:128, :], tiles[0][:, 1, :, :])
                sync.dma_start(agi[0, 128:CFULL, :], tiles[1][0:CC1, 1, :, :])
                sync.dma_start(agi[1, 0:128, :], tiles[0][:, 4, :, :])
                sync.dma_start(agi[1, 128:CFULL, :], tiles[1][0:CC1, 4, :, :])
                nc.gpsimd.collective_compute("AllGather", ALU.bypass, replica_groups=RG,
                                             ins=[agi[:, :, :]], outs=[ago[:, :, :]])
                sync.dma_start(tiles[0][:, 0, :, :], ago[bass.ds(sv_lo, 1), 0:128, :])
                sync.dma_start(tiles[1][0:CC1, 0, :, :], ago[bass.ds(sv_lo, 1), 128:CFULL, :])
                sync.dma_start(tiles[0][:, 5, :, :], ago[bass.ds(sv_hi, 1), 0:128, :])
                sync.dma_start(tiles[1][0:CC1, 5, :, :], ago[bass.ds(sv_hi, 1), 128:CFULL, :])
                vec.tensor_scalar_mul(tiles[0][:, 0, :, :], tiles[0][:, 0, :, :], mask_sb[:, 0:1])
                vec.tensor_scalar_mul(tiles[1][0:CC1, 0, :, :], tiles[1][0:CC1, 0, :, :], mask_sb[0:CC1, 0:1])
                vec.tensor_scalar_mul(tiles[0][:, 5, :, :], tiles[0][:, 5, :, :], mask_sb[:, 1:2])
                vec.tensor_scalar_mul(tiles[1][0:CC1, 5, :, :], tiles[1][0:CC1, 5, :, :], mask_sb[0:CC1, 1:2])

            # ---------------- c1 ----------------
            x2 = [big.tile([128, NPLANES, PL, PL], bf16, tag=f"bigslot{k}",
                           name=f"x2_{k}") for k in range(2)]
            nc.gpsimd.memset(x2[0], 0.0)
            nc.gpsimd.memset(x2[1], 0.0)
            conv_stage([x0, x1], w1_sb, b1_sb, ar_in[1], ar_out[1], x2, "c1",
                       post03=lambda: halo_exchange(x2, ag_in[1], ag_out[1]))

            # c2 weights into freed upconv-weight slots
            w2_sb = [wp.tile([128, 27, CFULL], bf16, tag=f"wb{k}", name=f"w2_sb{k}")
                     for k in range(2)]
            for k in range(2):
                sync.dma_start(w2_sb[k], w2d[k][:, :])

            # ---------------- c2 + flow-head V-term exchange ----------------
            cn = [big.tile([128, NPLANES, PL, PL], bf16, tag=f"bigslot{2 + k}",
                           name=f"cn_{k}") for k in range(2)]
            nc.gpsimd.memset(cn[0], 0.0)
            nc.gpsimd.memset(cn[1], 0.0)

            def c2_post03():
                # boundary partial sums for the flow head's stage 1:
                # V_bot = Wo_kz2 . Cn[plane 1] (to below), V_top = Wo_kz0 . Cn[plane 4]
                for ent, plane, kz in ((0, 1, 2), (1, 4, 0)):
                    for h in range(2):
                        pv = ps.tile([27, 16, 32], f32, tag="psacc",
                                     name=f"pv{ent}{h}")
                        for k in range(2):
                            rhs = cn[k][:, plane, 16 * h + 1: 16 * h + 17, 1:33]
                            nc.tensor.matmul(pv, wo_sb[k][:, kz, :], rhs,
                                             start=(k == 0), stop=(k == 1))
                        vs = scr.tile([128, 16, 32], bf16, tag=f"scrslot{h % 2}")
                        act.activation(vs[0:27, :, :], pv, AF.Copy)
                        sync.dma_start(
                            ag_in[2][ent, :, 512 * h: 512 * h + 512],
                            vs[0:27, :, :])
                nc.gpsimd.collective_compute("AllGather", ALU.bypass, replica_groups=RG,
                                             ins=[ag_in[2][:, :, :]],
                                             outs=[ag_out[2][:, :, :]])
                # also stream the Cn output while boundary planes are hot
                sync.dma_start(cn0d[:, 0:PLSZ], cn[0][:, 1, :, :])
                sync.dma_start(cn0d[:, 3 * PLSZ: 4 * PLSZ], cn[0][:, 4, :, :])
                sync.dma_start(cn1d[:, 0:PLSZ], cn[1][0:CC1, 1, :, :])
                sync.dma_start(cn1d[:, 3 * PLSZ: 4 * PLSZ], cn[1][0:CC1, 4, :, :])

            conv_stage([x2[0], x2[1]], w2_sb, b2_sb, ar_in[2], ar_out[2], cn, "c2",
                       post03=c2_post03)

            # rest of Cn output (interior planes)
            sync.dma_start(cn0d[:, PLSZ: 3 * PLSZ], cn[0][:, 2:4, :, :])
            sync.dma_start(cn1d[:, PLSZ: 3 * PLSZ], cn[1][0:CC1, 2:4, :, :])

            # V-term readback
            vrec_lo = cp.tile([27, 1024], bf16, tag="vrec0")
            vrec_hi = cp.tile([27, 1024], bf16, tag="vrec1")
            sync.dma_start(vrec_lo, ag_out[2][bass.ds(sv_lo, 1), :, :])
            sync.dma_start(vrec_hi, ag_out[2][bass.ds(sv_hi, 1), :, :])

            # ---------------- flow head ----------------
            yp = xp.tile([27, PD, PL, PL], bf16, tag="xslot1")
            nc.gpsimd.memset(yp, 0.0)
            flow_v = flowd[:, :].rearrange("p (a b c) -> p a b c", a=PD, b=32, c=32)

            def s2_plane(d):
                for h in range(2):
                    pt2 = ps.tile([3, 16, 32], f32, tag="psacc", name=f"fs2_{d}{h}")
                    first = True
                    for j in range(9):
                        ky, kx = j // 3, j % 3
                        rhs = yp[0:27, d, 16 * h + ky: 16 * h + ky + 16, kx: kx + 32]
                        nc.tensor.matmul(pt2, e_sb[:, 3 * j: 3 * j + 3], rhs,
                                         start=first, stop=(j == 8))
                        first = False
                    fo = scr.tile([3, 16, 32], f32, tag=f"fscr{h % 2}", name=f"fo{d}{h}")
                    act.activation(fo, pt2, AF.Identity, bias=bo_sb, scale=1.0)
                    sync.dma_start(flow_v[:, d, 16 * h: 16 * h + 16, :], fo)

            # stage-1 interior pass (planes 1, 2), weight-stationary
            pts1 = [ps.tile([27, 16, 32], f32, tag="psacc", name=f"fs1i{j}")
                    for j in range(4)]
            for kz in range(3):
                for k in range(2):
                    for j, (d, h) in enumerate(INT_BLOCKS):
                        rhs = cn[k][:, d + kz, 16 * h + 1: 16 * h + 17, 1:33]
                        nc.tensor.matmul(pts1[j], wo_sb[k][:, kz, :], rhs,
                                         start=(kz == 0 and k == 0),
                                         stop=(kz == 2 and k == 1))
            for j, (d, h) in enumerate(INT_BLOCKS):
                act.activation(yp[:, d, 16 * h + 1: 16 * h + 17, 1:33], pts1[j], AF.Copy)
            for d in (1, 2):
                s2_plane(d)

            # stage-1 boundary pass: own-kz terms + received V terms
            bnd_kzs = {0: (1, 2), 3: (0, 1)}
            pts1b = [ps.tile([27, 16, 32], f32, tag="psacc", name=f"fs1b{j}")
                     for j in range(4)]
            for j, (d, h) in enumerate(BND_BLOCKS):
                kzs = bnd_kzs[d]
                first = True
                for kz in kzs:
                    for k in range(2):
                        rhs = cn[k][:, d + kz, 16 * h + 1: 16 * h + 17, 1:33]
                        nc.tensor.matmul(pts1b[j], wo_sb[k][:, kz, :], rhs,
                                         start=first,
                                         stop=(kz == kzs[-1] and k == 1))
                        first = False
            for j, (d, h) in enumerate(BND_BLOCKS):
                act.activation(yp[:, d, 16 * h + 1: 16 * h + 17, 1:33], pts1b[j], AF.Copy)
            for d, vrec, mcol in ((0, vrec_lo, 0), (3, vrec_hi, 1)):
                for h in range(2):
                    vv = vrec[:, 512 * h: 512 * h + 512].rearrange(
                        "p (a b) -> p a b", a=16, b=32)
                    dst = yp[0:27, d, 16 * h + 1: 16 * h + 17, 1:33]
                    vec.scalar_tensor_tensor(dst, vv, mask_sb[0:27, mcol: mcol + 1],
                                             dst, ALU.mult, ALU.add)
            for d in (0, 3):
                s2_plane(d)

    nc.compile()
    return nc


# ---------------------------------------------------------------------------
# host-side input preparation
# ---------------------------------------------------------------------------

def host_prep(src, tgt, C, up_w, up_b, c1_w, c1_b, c2_w, c2_b, out_w, out_b):
    """Build the 8 per-core in_maps (numpy, bf16 compute precision)."""
    src = np.asarray(src); tgt = np.asarray(tgt); C = np.asarray(C)
    rows = cost_row_map()

    T1 = np.ascontiguousarray(np.transpose(np.asarray(c1_w), (1, 2, 3, 4, 0))).reshape(CFULL, 27, CFULL)
    w1k0 = T1[0:128].astype(BF16)
    w1k1 = np.zeros((128, 27, CFULL), np.float32)
    w1k1[0:64] = T1[155:219]                      # Cup channels at rows 0-63
    for t, r in rows.items():
        w1k1[r] = T1[128 + t]                     # cost channels at rows 64-90
    w1k1 = w1k1.astype(BF16)

    T2 = np.ascontiguousarray(np.transpose(np.asarray(c2_w), (1, 2, 3, 4, 0))).reshape(CFULL, 27, CFULL)
    w2k0 = T2[0:128].astype(BF16)
    w2k1 = np.zeros((128, 27, CFULL), np.float32)
    w2k1[0:CC1] = T2[128:219]
    w2k1 = w2k1.astype(BF16)

    # upconv: wc[o, ci, k] = up_w[ci, o, 3-k]; Wu[ci, p, t, o] = wc[o, ci, 2t+p]
    wc = np.flip(np.transpose(np.asarray(up_w), (1, 0, 2, 3, 4)), axis=(2, 3, 4))
    wcr = np.transpose(wc, (1, 2, 3, 4, 0))       # [ci, kz, ky, kx, o]
    Wu = np.zeros((512, 8, 8, 64), np.float32)
    for p in range(8):
        pz, py, px = p >> 2 & 1, p >> 1 & 1, p & 1
        for t in range(8):
            tz, ty, tx = t >> 2 & 1, t >> 1 & 1, t & 1
            Wu[0:411, p, t, :] = wcr[:, 2 * tz + pz, 2 * ty + py, 2 * tx + px, :]
    wu = [Wu[128 * k: 128 * k + 128].reshape(128, 4096).astype(BF16) for k in range(4)]

    To = np.transpose(np.asarray(out_w), (1, 2, 3, 4, 0)).reshape(CFULL, 3, 27)
    wo0 = To[0:128].reshape(128, 81).astype(BF16)
    wo1 = np.zeros((128, 3, 27), np.float32)
    wo1[0:CC1] = To[128:219]
    wo1 = wo1.reshape(128, 81).astype(BF16)
    ec = np.zeros((27, 27), np.float32)
    for j in range(9):
        for o in range(3):
            ec[3 * j + o, 3 * j + o] = 1.0
    ec = ec.astype(BF16)

    red = np.zeros((128, 32), np.float32)
    red[0:64, 0] = 1.0 / 64
    red[64:128, 1] = 1.0 / 64
    red = red.astype(BF16)

    c1_b = np.asarray(c1_b); c2_b = np.asarray(c2_b)
    b1c0 = c1_b[0:128].reshape(128, 1).astype(np.float32)
    b1c1 = np.zeros((128, 1), np.float32); b1c1[0:CC1, 0] = c1_b[128:219]
    b2c0 = c2_b[0:128].reshape(128, 1).astype(np.float32)
    b2c1 = np.zeros((128, 1), np.float32); b2c1[0:CC1, 0] = c2_b[128:219]
    bu = np.asarray(up_b).reshape(64, 1).astype(np.float32)
    bo = np.asarray(out_b).reshape(3, 1).astype(np.float32)

    shared = {
        "wu0": wu[0], "wu1": wu[1], "wu2": wu[2], "wu3": wu[3],
        "w1k0": w1k0.reshape(128, 27 * CFULL), "w1k1": w1k1.reshape(128, 27 * CFULL),
        "w2k0": w2k0.reshape(128, 27 * CFULL), "w2k1": w2k1.reshape(128, 27 * CFULL),
        "wo0": wo0, "wo1": wo1, "ec": ec, "red": red,
        "b1c0": b1c0, "b1c1": b1c1, "b2c0": b2c0, "b2c1": b2c1,
        "bu": bu, "bo": bo,
    }

    ts = np.concatenate([tgt[0], src[0]], axis=0).astype(BF16)  # [128, 32, 32, 32]
    Cb = C[0].astype(BF16)                                      # [411, 16, 16, 16]

    in_maps = []
    for k in range(NCORES):
        x0 = np.zeros((128, NPLANES, PL, PL), BF16)
        zlo = 4 * k - 1
        for zp in range(NPLANES):
            z = zlo + zp
            if 0 <= z < D:
                x0[:, zp, 1:33, 1:33] = ts[:, z]
        cs = []
        for kc in range(4):
            a = np.zeros((128, 4, 18, 18), BF16)
            for ip in range(4):
                z = 2 * k - 1 + ip
                if 0 <= z < 16:
                    c0 = 128 * kc
                    n = min(128, 411 - c0)
                    if n > 0:
                        a[0:n, ip, 1:17, 1:17] = Cb[c0:c0 + n, z]
            cs.append(a.reshape(128, 4 * CP))
        mask = np.zeros((128, 2), np.float32)
        mask[:, 0] = 1.0 if k > 0 else 0.0
        mask[:, 1] = 1.0 if k < 7 else 0.0
        hidx = np.array([[2 * (k - 1) + 1 if k > 0 else 0,
                          2 * (k + 1) if k < 7 else 0]], np.uint32)
        m = dict(shared)
        m["x0"] = x0.reshape(128, NPLANES * PLSZ)
        for kc in range(4):
            m[f"cs{kc}"] = cs[kc]
        m["mask"] = mask
        m["hidx"] = hidx
        in_maps.append(m)
    return in_maps


def assemble_outputs(results):
    Cn = np.zeros((219, 32, 32, 32), np.float32)
    out = np.zeros((3, 32, 32, 32), np.float32)
    for k, r in enumerate(results):
        cnb = np.concatenate([np.asarray(r["cn0"]).astype(np.float32),
                              np.asarray(r["cn1"]).astype(np.float32)], axis=0)
        cnb = cnb.reshape(CFULL, PD, PL, PL)[:, :, 1:33, 1:33]
        Cn[:, 4 * k: 4 * k + 4] = cnb
        out[:, 4 * k: 4 * k + 4] = np.asarray(r["flow"]).reshape(3, PD, 32, 32)
    return Cn[None], out[None]


_PROG = None


def _get_prog():
    global _PROG
    if _PROG is None:
        _PROG = build_program()
    return _PROG


def kernel(**inputs):
    nc = _get_prog()
    in_maps = host_prep(**inputs)
    from concourse.bass_utils import run_bass_kernel_spmd
    res = run_bass_kernel_spmd(nc, in_maps, core_ids=list(range(NCORES)))
    return assemble_outputs(res.results)


# revision 12
# speedup vs baseline: 1.1294x; 1.1294x over previous
"""Trainium2 Bass kernel for nn_Decoder_76836964926387.

Decoder block: upconv (ConvTranspose3d k4 s2 p1) + instance-norm + leaky,
3x3x3 correlation volume, concat, two ConvInsBlocks (3^3 conv + IN + leaky),
and a 3-channel flow head. Returns (Cn, out).

Distribution: depth-axis sharding across 8 NeuronCores (4 of 32 z-planes per
core). Instance-norm statistics via AllReduce; halo planes via AllGather with
dynamically-indexed readback; conv compute as bf16 matmuls accumulating in
PSUM (fp32). Convs run as weight-stationary interior/boundary passes so the
halo AllGathers hide under interior compute; the flow head exchanges
precomputed 27-channel boundary partial sums instead of full 219-channel
halo planes.

Self-contained: all shapes/shardings hardcoded for the fixed problem size.
"""
import sys
import os

sys.path.insert(0, '/opt/trn_rl_repo')

import numpy as np
import ml_dtypes

import concourse.bass as bass
import concourse.bacc as bacc
import concourse.tile as tile
import concourse.mybir as mybir

BF16 = ml_dtypes.bfloat16
dt = mybir.dt
AF = mybir.ActivationFunctionType
ALU = mybir.AluOpType

NCORES = 8
D = 32              # full volume depth/height/width
PD = 4              # own z-planes per core
PL = 34             # padded plane edge
PLSZ = PL * PL      # 1156
NPLANES = 6         # own 4 + 2 halo
CFULL = 219         # x / conv channels
CC0, CC1 = 128, 91  # channel chunks
NTOT = float(D * D * D)  # instance-norm element count
EPS = 1e-5
ALPHA = 0.1
CP = 18 * 18        # padded C-slab plane (16+2)^2

RG = [list(range(NCORES))]

INT_BLOCKS = [(1, 0), (1, 1), (2, 0), (2, 1)]
BND_BLOCKS = [(0, 0), (0, 1), (3, 0), (3, 1)]


def tap_idx(dz, dy, dx):
    return (dz + 1) * 9 + (dy + 1) * 3 + (dx + 1)


# correlation tap pairing: lo tap, hi = lo with dx/dy +1 instead of -1.
# 'A' pairs share the +2 pre-shifted src copy, 'B' pairs the +68 one.
PAIRS = (
    [((dz, dy, -1), 'A') for dz in (-1, 0, 1) for dy in (-1, 0, 1)]
    + [((dz, -1, 0), 'B') for dz in (-1, 0, 1)]
)
SINGLES = [(-1, 0, 0), (1, 0, 0), (0, 0, 0)]


def _pair_hi(lo, kind):
    dz, dy, dx = lo
    return (dz, dy, 1) if kind == 'A' else (dz, 1, 0)


def cost_row_map():
    """x-tile1 partition row for each corr tap (rows 64..90)."""
    rows = {}
    for j, (lo, kind) in enumerate(PAIRS):
        rows[tap_idx(*lo)] = 64 + 2 * j
        rows[tap_idx(*_pair_hi(lo, kind))] = 64 + 2 * j + 1
    for i, t in enumerate(SINGLES):
        rows[tap_idx(*t)] = 88 + i
    return rows


def off3(dz, dy, dx):
    return dz * PLSZ + dy * PL + dx


# ---------------------------------------------------------------------------
# program builder
# ---------------------------------------------------------------------------

def build_program():
    nc = bacc.Bacc("TRN2", target_bir_lowering=False, debug=False,
                   num_devices=NCORES)

    f32, bf16, u32 = dt.float32, dt.bfloat16, dt.uint32

    # ---- kernel I/O ----
    x0d = nc.dram_tensor("x0", [128, NPLANES * PLSZ], bf16, kind="ExternalInput")
    csd = [nc.dram_tensor(f"cs{k}", [128, 4 * CP], bf16, kind="ExternalInput") for k in range(4)]
    wud = [nc.dram_tensor(f"wu{k}", [128, 4096], bf16, kind="ExternalInput") for k in range(4)]
    w1d = [nc.dram_tensor(f"w1k{k}", [128, 27 * CFULL], bf16, kind="ExternalInput") for k in range(2)]
    w2d = [nc.dram_tensor(f"w2k{k}", [128, 27 * CFULL], bf16, kind="ExternalInput") for k in range(2)]
    wod = [nc.dram_tensor(f"wo{k}", [128, 81], bf16, kind="ExternalInput") for k in range(2)]
    ed = nc.dram_tensor("ec", [27, 27], bf16, kind="ExternalInput")
    redd = nc.dram_tensor("red", [128, 32], bf16, kind="ExternalInput")
    b1d = [nc.dram_tensor(f"b1c{k}", [128, 1], f32, kind="ExternalInput") for k in range(2)]
    b2d = [nc.dram_tensor(f"b2c{k}", [128, 1], f32, kind="ExternalInput") for k in range(2)]
    bud = nc.dram_tensor("bu", [64, 1], f32, kind="ExternalInput")
    bod = nc.dram_tensor("bo", [3, 1], f32, kind="ExternalInput")
    maskd = nc.dram_tensor("mask", [128, 2], f32, kind="ExternalInput")
    hidxd = nc.dram_tensor("hidx", [1, 2], u32, kind="ExternalInput")

    cn0d = nc.dram_tensor("cn0", [128, PD * PLSZ], bf16, kind="ExternalOutput")
    cn1d = nc.dram_tensor("cn1", [CC1, PD * PLSZ], bf16, kind="ExternalOutput")
    flowd = nc.dram_tensor("flow", [3, PD * 1024], f32, kind="ExternalOutput")

    # ---- collective bounce buffers ----
    ag_shapes = [(2, CC1, PLSZ), (2, CFULL, PLSZ), (2, 27, 1024)]
    ag_in = [nc.dram_tensor(f"agi{i}", list(s), bf16, kind="Internal")
             for i, s in enumerate(ag_shapes)]
    ag_out = [nc.dram_tensor(f"ago{i}", [2 * NCORES] + list(s[1:]), bf16,
                             kind="Internal", addr_space="Shared")
              for i, s in enumerate(ag_shapes)]
    ar_in = [nc.dram_tensor("ari0", [64, 2], f32, kind="Internal"),
             [nc.dram_tensor(f"ari1_{m}", [128, 2], f32, kind="Internal") for m in range(2)],
             [nc.dram_tensor(f"ari2_{m}", [128, 2], f32, kind="Internal") for m in range(2)]]
    ar_out = [nc.dram_tensor("aro0", [64, 2], f32, kind="Internal", addr_space="Shared"),
              [nc.dram_tensor(f"aro1_{m}", [128, 2], f32, kind="Internal",
                              addr_space="Shared") for m in range(2)],
              [nc.dram_tensor(f"aro2_{m}", [128, 2], f32, kind="Internal",
                              addr_space="Shared") for m in range(2)]]

    with tile.TileContext(nc) as tc:
        with tc.tile_pool(name="cp", bufs=1) as cp, \
             tc.tile_pool(name="wp", bufs=1) as wp, \
             tc.tile_pool(name="xp", bufs=1) as xp, \
             tc.tile_pool(name="big", bufs=1) as big, \
             tc.tile_pool(name="prod", bufs=1) as prod, \
             tc.tile_pool(name="rawp", bufs=1) as rawp, \
             tc.tile_pool(name="scr", bufs=1) as scr, \
             tc.tile_pool(name="stat", bufs=1) as stat, \
             tc.tile_pool(name="ps", bufs=8, space="PSUM") as ps:

            sync = nc.sync
            act = nc.scalar
            vec = nc.vector

            # ---------------- consts ----------------
            hidx_sb = cp.tile([1, 2], u32, tag="hidx")
            sync.dma_start(hidx_sb, hidxd[:, :])
            mask_sb = cp.tile([128, 2], f32, tag="mask")
            sync.dma_start(mask_sb, maskd[:, :])
            red_sb = cp.tile([128, 32], bf16, tag="red")
            sync.dma_start(red_sb, redd[:, :])
            e_sb = cp.tile([27, 27], bf16, tag="ec")
            sync.dma_start(e_sb, ed[:, :])
            wo_sb = [cp.tile([128, 3, 27], bf16, tag=f"wo{k}", name=f"wo_sb{k}") for k in range(2)]
            for k in range(2):
                sync.dma_start(wo_sb[k], wod[k][:, :])
            b1_sb = [cp.tile([128, 1], f32, tag=f"b1_{k}", name=f"b1_sb{k}") for k in range(2)]
            b2_sb = [cp.tile([128, 1], f32, tag=f"b2_{k}", name=f"b2_sb{k}") for k in range(2)]
            for k in range(2):
                sync.dma_start(b1_sb[k], b1d[k][:, :])
                sync.dma_start(b2_sb[k], b2d[k][:, :])
            bu_sb = cp.tile([64, 1], f32, tag="bu")
            sync.dma_start(bu_sb, bud[:, :])
            bo_sb = cp.tile([3, 1], f32, tag="bo")
            sync.dma_start(bo_sb, bod[:, :])
            eps_sb = cp.tile([128, 1], f32, tag="epsc")
            vec.memset(eps_sb, EPS)

            # halo indices -> registers
            r_lo = nc.alloc_registers("r_lo")
            nc.regs_load(r_lo, hidx_sb[0:1, 0:1])
            sv_lo = nc.snap(r_lo, donate=True)
            r_hi = nc.alloc_registers("r_hi")
            nc.regs_load(r_hi, hidx_sb[0:1, 1:2])
            sv_hi = nc.snap(r_hi, donate=True)

            # ---------------- input / weight loads ----------------
            cs_sb = [wp.tile([128, 4, 18, 18], bf16, tag=f"wa{k}", name=f"cs_sb{k}") for k in range(4)]
            for k in range(4):
                sync.dma_start(cs_sb[k], csd[k][:, :])
            wu_sb = [wp.tile([128, 8, 8, 64], bf16, tag=f"wb{k}", name=f"wu_sb{k}") for k in range(4)]
            for k in range(4):
                sync.dma_start(wu_sb[k], wud[k][:, :])

            x0 = xp.tile([128, NPLANES, PL, PL], bf16, tag="xslot0")
            sync.dma_start(x0, x0d[:, :])
            x1 = xp.tile([128, NPLANES, PL, PL], bf16, tag="xslot1")
            nc.gpsimd.memset(x1, 0.0)

            # c1 weights: chunk0 borrows the (late-used) cn1 big slot; chunk1
            # reuses the C-slab slot that frees after the upconv's kc0 sweep.
            w1_sb = [big.tile([128, 27, CFULL], bf16, tag="bigslot3", name="w1_sb0"),
                     wp.tile([128, 27, CFULL], bf16, tag="wa0", name="w1_sb1")]
            sync.dma_start(w1_sb[0], w1d[0][:, :])
            sync.dma_start(w1_sb[1], w1d[1][:, :])

            # corr pre-shifted src copies (emitted early so DMA queues warm)
            tgt2 = big.tile([128, NPLANES, PL, PL], bf16, tag="bigslot0")
            srcA = big.tile([128, NPLANES, PL, PL], bf16, tag="bigslot1")
            srcB = big.tile([128, NPLANES, PL, PL], bf16, tag="bigslot2")
            t2f = tgt2.rearrange("p a b c -> p (a b c)")
            sAf = srcA.rearrange("p a b c -> p (a b c)")
            sBf = srcB.rearrange("p a b c -> p (a b c)")
            x0f = x0.rearrange("p a b c -> p (a b c)")
            FS = NPLANES * PLSZ
            nc.gpsimd.memset(srcA[64:128, :, :, :], 0.0)
            nc.gpsimd.memset(srcB[64:128, :, :, :], 0.0)
            sync.dma_start(t2f[0:64, :], x0f[0:64, :])
            sync.dma_start(t2f[64:128, :], x0f[0:64, :])
            sync.dma_start(sAf[0:64, :], x0f[64:128, :])
            sync.dma_start(sAf[64:128, 0:FS - 2], x0f[64:128, 2:FS])
            sync.dma_start(sBf[0:64, :], x0f[64:128, :])
            sync.dma_start(sBf[64:128, 0:FS - 68], x0f[64:128, 68:FS])

            # ------- phase 1: upconv + corr, interleaved on the PE -------
            # The corr reductions are tiny matmuls gated by PSUM-slot recycling
            # (evac latency); interleaving them into the upconv stream keeps
            # the PE dense.
            cup_raw = rawp.tile([64, 8, 512], bf16, tag="rawslot0")
            sums_u = stat.tile([64, 16], f32, tag="sumsu")
            QLO, QHI = 35, 4 * PLSZ - 35
            cost_dmas = []
            ALL_BLOCKS = INT_BLOCKS + BND_BLOCKS

            corr_units = []
            prod_tiles = {}

            def emit_products(g):
                pts = []
                for gi in range(2):
                    j = 2 * g + gi
                    lo, kind = PAIRS[j]
                    pt = prod.tile([128, PD, PL, PL], bf16,
                                   tag=f"prodslot{(2 * g + gi) % 3}", name=f"pp{g}_{gi}")
                    ptf = pt.rearrange("p a b c -> p (a b c)")
                    srcf = sAf if kind == 'A' else sBf
                    d0 = off3(*lo)
                    vec.tensor_mul(ptf[:, QLO:QHI],
                                   t2f[:, PLSZ + QLO: PLSZ + QHI],
                                   srcf[:, PLSZ + QLO + d0: PLSZ + QHI + d0])
                    pts.append(pt)
                prod_tiles[g] = pts

            def emit_pair_block(g, b):
                d, h = ALL_BLOCKS[b]
                pt_ps = ps.tile([128, 16, 32], f32, tag="psacc", name=f"cr{g}_{b}")
                for gi in range(2):
                    rhs = prod_tiles[g][gi][:, d, 16 * h + 1: 16 * h + 17, 1:33]
                    nc.tensor.matmul(pt_ps[32 * gi: 32 * gi + 32, :, :],
                                     red_sb, rhs,
                                     tile_position=(0, 32 * gi))
                st = scr.tile([128, 16, 32], bf16, tag=f"cstg{b % 4}", name=f"cs{g}_{b}")
                act.activation(st[0:34, :, :], pt_ps[0:34, :, :], AF.Copy)
                for gi in range(2):
                    j = 2 * g + gi
                    cost_dmas.append((st, 32 * gi, 64 + 2 * j, 2, d, h))

            def emit_single_products(i):
                tp = SINGLES[i]
                pt = prod.tile([128, PD, PL, PL], bf16, tag=f"prodslot{i % 3}",
                               name=f"sp{i}")
                ptf = pt.rearrange("p a b c -> p (a b c)")
                d0 = off3(*tp)
                vec.tensor_mul(ptf[0:64, QLO:QHI],
                               t2f[0:64, PLSZ + QLO: PLSZ + QHI],
                               sAf[0:64, PLSZ + QLO + d0: PLSZ + QHI + d0])
                prod_tiles[('s', i)] = pt

            def emit_single_block(i, b):
                d, h = ALL_BLOCKS[b]
                pt_ps = ps.tile([128, 16, 32], f32, tag="psacc", name=f"sr{i}_{b}")
                rhs = prod_tiles[('s', i)][0:64, d, 16 * h + 1: 16 * h + 17, 1:33]
                nc.tensor.matmul(pt_ps[0:32, :, :], red_sb[0:64, 0:32], rhs)
                st = scr.tile([128, 16, 32], bf16, tag=f"cstg{b % 4}", name=f"ss{i}_{b}")
                act.activation(st[0:1, :, :], pt_ps[0:1, :, :], AF.Copy)
                cost_dmas.append((st, 0, 88 + i, 1, d, h))

            for g in range(6):
                corr_units.append(lambda g=g: emit_products(g))
                for b in range(8):
                    corr_units.append(lambda g=g, b=b: emit_pair_block(g, b))
            for i in range(3):
                corr_units.append(lambda i=i: emit_single_products(i))
                for b in range(8):
                    corr_units.append(lambda i=i, b=b: emit_single_block(i, b))
            cu = iter(corr_units)

            def pop_units(n):
                for _ in range(n):
                    u = next(cu, None)
                    if u is not None:
                        u()

            # products for the first group before the PE stream begins
            pop_units(1)
            for half in range(2):
                psu = [ps.tile([64, 2, 16, 16], f32, tag="psacc", name=f"psu{half}_{j}")
                       for j in range(4)]
                for k in range(4):
                    for j in range(4):
                        p = 4 * half + j
                        pz, py, px = p >> 2 & 1, p >> 1 & 1, p & 1
                        for t in range(8):
                            tz, ty, tx = t >> 2 & 1, t >> 1 & 1, t & 1
                            rhs = cs_sb[k][:, tz + pz: tz + pz + 2,
                                           ty + py: ty + py + 16,
                                           tx + px: tx + px + 16]
                            nc.tensor.matmul(psu[j], wu_sb[k][:, p, t, :], rhs,
                                             start=(k == 0 and t == 0),
                                             stop=(k == 3 and t == 7))
                        pop_units(3 if (k % 2 == 0) else 2)
                for j in range(4):
                    p = 4 * half + j
                    act.activation(cup_raw[:, p, :].rearrange("p (a b c) -> p a b c", a=2, b=16, c=16),
                                   psu[j], AF.Identity, bias=bu_sb, scale=1.0,
                                   accum_out=sums_u[:, p: p + 1])
                    sq = scr.tile([128, 16, 32], bf16, tag=f"scrslot{j % 2}")
                    act.activation(sq[0:64, 0:16, 0:32].rearrange("p a b -> p (a b)"),
                                   cup_raw[:, p, :], AF.Square,
                                   accum_out=sums_u[:, 8 + p: 9 + p])
            pop_units(1000)
            for st, srow, drow, n, d, h in cost_dmas:
                sync.dma_start(x1[drow:drow + n, d + 1, 16 * h + 1: 16 * h + 17, 1:33],
                               st[srow:srow + n, :, :])

            # ---------------- AR#1: upconv instance-norm stats --------------
            st_u = stat.tile([64, 2], f32, tag="aru")
            vec.reduce_sum(st_u[:, 0:1], sums_u[:, 0:8], axis=mybir.AxisListType.X)
            vec.reduce_sum(st_u[:, 1:2], sums_u[:, 8:16], axis=mybir.AxisListType.X)
            sync.dma_start(ar_in[0][:, :], st_u)
            nc.gpsimd.collective_compute("AllReduce", ALU.add, replica_groups=RG,
                                         ins=[ar_in[0][:, :]], outs=[ar_out[0][:, :]])
            g_u = stat.tile([64, 2], f32, tag="gu")
            sync.dma_start(g_u, ar_out[0][:, :])

            def norm_coeffs(g, n, tagp):
                m = stat.tile([n, 1], f32, tag=tagp + "m", name=tagp + "m")
                vec.tensor_scalar_mul(m, g[0:n, 0:1], 1.0 / NTOT)
                q = stat.tile([n, 1], f32, tag=tagp + "q", name=tagp + "q")
                vec.tensor_scalar_mul(q, g[0:n, 1:2], 1.0 / NTOT)
                v = stat.tile([n, 1], f32, tag=tagp + "v", name=tagp + "v")
                vec.scalar_tensor_tensor(v, m, -1.0, m, ALU.mult, ALU.mult)
                vec.tensor_add(v, q, v)
                u = stat.tile([n, 1], f32, tag=tagp + "u", name=tagp + "u")
                act.activation(u, v, AF.Sqrt, bias=eps_sb[0:n, :], scale=1.0)
                s = stat.tile([n, 1], f32, tag=tagp + "s", name=tagp + "s")
                vec.reciprocal(s, u)
                bb = stat.tile([n, 1], f32, tag=tagp + "b", name=tagp + "b")
                vec.scalar_tensor_tensor(bb, m, -1.0, s, ALU.mult, ALU.mult)
                return s, bb

            s_u, b_u = norm_coeffs(g_u, 64, "u")

            # upconv norm + leaky -> x1 rows 0:64 (strided parity writes)
            for p in range(8):
                pz, py, px = p >> 2 & 1, p >> 1 & 1, p & 1
                a_t = scr.tile([128, 16, 32], bf16, tag=f"scrslot{p % 2}")
                a_v = a_t[0:64, 0:16, 0:32].rearrange("p a b -> p (a b)")
                act.activation(a_v, cup_raw[:, p, :], AF.Identity,
                               bias=b_u, scale=s_u)
                a_r = a_t[0:64, 0:16, 0:32].rearrange("p a b -> p (a b)").rearrange(
                    "p (a b c) -> p a b c", a=2, b=16, c=16)
                for iz in range(2):
                    dst = x1[0:64, 1 + pz + 2 * iz, 1 + py: 33: 2, 1 + px: 33: 2]
                    vec.scalar_tensor_tensor(dst, a_r[:, iz, :, :], ALPHA,
                                             a_r[:, iz, :, :], ALU.mult, ALU.max)

            # ---------------- AG#1: x1 halo planes ----------------
            sync.dma_start(ag_in[0][0, :, :], x1[0:CC1, 1, :, :])
            sync.dma_start(ag_in[0][1, :, :], x1[0:CC1, 4, :, :])
            nc.gpsimd.collective_compute("AllGather", ALU.bypass, replica_groups=RG,
                                         ins=[ag_in[0][:, :, :]], outs=[ag_out[0][:, :, :]])
            sync.dma_start(x1[0:CC1, 0, :, :], ag_out[0][bass.ds(sv_lo, 1), :, :])
            sync.dma_start(x1[0:CC1, 5, :, :], ag_out[0][bass.ds(sv_hi, 1), :, :])
            vec.tensor_scalar_mul(x1[0:CC1, 0, :, :], x1[0:CC1, 0, :, :], mask_sb[0:CC1, 0:1])
            vec.tensor_scalar_mul(x1[0:CC1, 5, :, :], x1[0:CC1, 5, :, :], mask_sb[0:CC1, 1:2])

            # ---------------- generic conv + IN + leaky stage ----------------
            def conv_stage(xin_tiles, w_tiles, b_tiles, ar_i, ar_o, out_tiles,
                           tagp, post03=None):
                """Weight-stationary 3^3 conv passes + IN stats + leaky.

                Four passes: (mc0,int), (mc1,int), (mc0,bnd), (mc1,bnd) — the
                boundary passes run last so the previous stage's halo exchange
                hides under interior compute. post03() fires after output
                planes 0 and 3 are normalized (to kick the next exchange).
                """
                raws = [rawp.tile([128, PD, 32, 32], bf16, tag=f"rawslot{mc}",
                                  name=f"{tagp}raw{mc}") for mc in range(2)]
                sums = [stat.tile([128, 16], f32, tag=tagp + f"sums{mc}",
                                  name=f"{tagp}sums{mc}") for mc in range(2)]
                passes = [(0, INT_BLOCKS, 0), (0, BND_BLOCKS, 4),
                          (1, INT_BLOCKS, 0), (1, BND_BLOCKS, 4)]
                coeffs = [None, None]
                for mc, blks, coff in passes:
                    mlen = CC0 if mc == 0 else CC1
                    pts = [ps.tile([128, 16, 32], f32, tag="psacc",
                                   name=f"{tagp}ps{mc}{coff}{j}") for j in range(4)]
                    for t in range(27):
                        tz, ty, tx = t // 9, (t // 3) % 3, t % 3
                        for k in range(2):
                            w_ap = w_tiles[k][:, t, 128 * mc: 128 * mc + mlen]
                            for j, (d, h) in enumerate(blks):
                                rhs = xin_tiles[k][:, d + tz,
                                                   16 * h + ty: 16 * h + ty + 16,
                                                   tx: tx + 32]
                                nc.tensor.matmul(
                                    pts[j][0:mlen, :, :], w_ap, rhs,
                                    start=(t == 0 and k == 0),
                                    stop=(t == 26 and k == 1))
                    for j, (d, h) in enumerate(blks):
                        act.activation(raws[mc][0:mlen, d, 16 * h: 16 * h + 16, :],
                                       pts[j][0:mlen, :, :], AF.Identity,
                                       bias=b_tiles[mc][0:mlen, :], scale=1.0,
                                       accum_out=sums[mc][0:mlen, coff + j: coff + j + 1])
                        sq = scr.tile([128, 16, 32], bf16, tag=f"scrslot{j % 2}")
                        act.activation(sq[0:mlen, :, :],
                                       raws[mc][0:mlen, d, 16 * h: 16 * h + 16, :],
                                       AF.Square,
                                       accum_out=sums[mc][0:mlen, 8 + coff + j: 9 + coff + j])
                    if coff == 4:
                        # this mc's stats are complete: allreduce them while the
                        # other chunk's passes still run on the PE
                        stt = stat.tile([128, 2], f32, tag=tagp + f"st{mc}",
                                        name=f"{tagp}st{mc}")
                        vec.reduce_sum(stt[0:mlen, 0:1], sums[mc][0:mlen, 0:8],
                                       axis=mybir.AxisListType.X)
                        vec.reduce_sum(stt[0:mlen, 1:2], sums[mc][0:mlen, 8:16],
                                       axis=mybir.AxisListType.X)
                        sync.dma_start(ar_i[mc][0:mlen, :], stt[0:mlen, :])
                        nc.gpsimd.collective_compute(
                            "AllReduce", ALU.add, replica_groups=RG,
                            ins=[ar_i[mc][0:mlen, :]], outs=[ar_o[mc][0:mlen, :]])
                        g = stat.tile([128, 2], f32, tag=tagp + f"g{mc}",
                                      name=f"{tagp}g{mc}")
                        sync.dma_start(g[0:mlen, :], ar_o[mc][0:mlen, :])
                        coeffs[mc] = norm_coeffs(g, mlen, tagp + f"c{mc}")
                for d in (0, 3, 1, 2):
                    for mc in range(2):
                        mlen = CC0 if mc == 0 else CC1
                        s, bb = coeffs[mc]
                        a_t = prod.tile([128, 32, 32], bf16, tag=f"prodslot{mc % 2}",
                                        name=f"{tagp}a{mc}{d}")
                        act.activation(a_t[0:mlen, :, :], raws[mc][0:mlen, d, :, :],
                                       AF.Identity, bias=bb, scale=s)
                        vec.scalar_tensor_tensor(
                            out_tiles[mc][0:mlen, d + 1, 1:33, 1:33],
                            a_t[0:mlen, :, :], ALPHA, a_t[0:mlen, :, :],
                            ALU.mult, ALU.max)
                    if d == 3 and post03 is not None:
                        post03()

            def halo_exchange(tiles, agi, ago):
                sync.dma_start(agi[0, 0:128, :], tiles[0][:, 1, :, :])
                sync.dma_start(agi[0, 128:CFULL, :], tiles[1][0:CC1, 1, :, :])
                sync.dma_start(agi[1, 0:128, :], tiles[0][:, 4, :, :])
                sync.dma_start(agi[1, 128:CFULL, :], tiles[1][0:CC1, 4, :, :])
                nc.gpsimd.collective_compute("AllGather", ALU.bypass, replica_groups=RG,
                                             ins=[agi[:, :, :]], outs=[ago[:, :, :]])
                sync.dma_start(tiles[0][:, 0, :, :], ago[bass.ds(sv_lo, 1), 0:128, :])
                sync.dma_start(tiles[1][0:CC1, 0, :, :], ago[bass.ds(sv_lo, 1), 128:CFULL, :])
                sync.dma_start(tiles[0][:, 5, :, :], ago[bass.ds(sv_hi, 1), 0:128, :])
                sync.dma_start(tiles[1][0:CC1, 5, :, :], ago[bass.ds(sv_hi, 1), 128:CFULL, :])
                vec.tensor_scalar_mul(tiles[0][:, 0, :, :], tiles[0][:, 0, :, :], mask_sb[:, 0:1])
                vec.tensor_scalar_mul(tiles[1][0:CC1, 0, :, :], tiles[1][0:CC1, 0, :, :], mask_sb[0:CC1, 0:1])
                vec.tensor_scalar_mul(tiles[0][:, 5, :, :], tiles[0][:, 5, :, :], mask_sb[:, 1:2])
                vec.tensor_scalar_mul(tiles[1][0:CC1, 5, :, :], tiles[1][0:CC1, 5, :, :], mask_sb[0:CC1, 1:2])

            # ---------------- c1 ----------------
            x2 = [big.tile([128, NPLANES, PL, PL], bf16, tag=f"bigslot{k}",
                           name=f"x2_{k}") for k in range(2)]
            nc.gpsimd.memset(x2[0], 0.0)
            nc.gpsimd.memset(x2[1], 0.0)
            conv_stage([x0, x1], w1_sb, b1_sb, ar_in[1], ar_out[1], x2, "c1",
                       post03=lambda: halo_exchange(x2, ag_in[1], ag_out[1]))

            # c2 weights into freed upconv-weight slots
            w2_sb = [wp.tile([128, 27, CFULL], bf16, tag=f"wb{k}", name=f"w2_sb{k}")
                     for k in range(2)]
            for k in range(2):
                sync.dma_start(w2_sb[k], w2d[k][:, :])

            # ---------------- c2 + flow-head V-term exchange ----------------
            cn = [big.tile([128, NPLANES, PL, PL], bf16, tag=f"bigslot{2 + k}",
                           name=f"cn_{k}") for k in range(2)]
            nc.gpsimd.memset(cn[0], 0.0)
            nc.gpsimd.memset(cn[1], 0.0)

            def c2_post03():
                # boundary partial sums for the flow head's stage 1:
                # V_bot = Wo_kz2 . Cn[plane 1] (to below), V_top = Wo_kz0 . Cn[plane 4]
                for ent, plane, kz in ((0, 1, 2), (1, 4, 0)):
                    for h in range(2):
                        pv = ps.tile([27, 16, 32], f32, tag="psacc",
                                     name=f"pv{ent}{h}")
                        for k in range(2):
                            rhs = cn[k][:, plane, 16 * h + 1: 16 * h + 17, 1:33]
                            nc.tensor.matmul(pv, wo_sb[k][:, kz, :], rhs,
                                             start=(k == 0), stop=(k == 1))
                        vs = scr.tile([128, 16, 32], bf16, tag=f"scrslot{h % 2}")
                        act.activation(vs[0:27, :, :], pv, AF.Copy)
                        sync.dma_start(
                            ag_in[2][ent, :, 512 * h: 512 * h + 512],
                            vs[0:27, :, :])
                nc.gpsimd.collective_compute("AllGather", ALU.bypass, replica_groups=RG,
                                             ins=[ag_in[2][:, :, :]],
                                             outs=[ag_out[2][:, :, :]])
                # also stream the Cn output while boundary planes are hot
                sync.dma_start(cn0d[:, 0:PLSZ], cn[0][:, 1, :, :])
                sync.dma_start(cn0d[:, 3 * PLSZ: 4 * PLSZ], cn[0][:, 4, :, :])
                sync.dma_start(cn1d[:, 0:PLSZ], cn[1][0:CC1, 1, :, :])
                sync.dma_start(cn1d[:, 3 * PLSZ: 4 * PLSZ], cn[1][0:CC1, 4, :, :])

            conv_stage([x2[0], x2[1]], w2_sb, b2_sb, ar_in[2], ar_out[2], cn, "c2",
                       post03=c2_post03)

            # rest of Cn output (interior planes)
            sync.dma_start(cn0d[:, PLSZ: 3 * PLSZ], cn[0][:, 2:4, :, :])
            sync.dma_start(cn1d[:, PLSZ: 3 * PLSZ], cn[1][0:CC1, 2:4, :, :])

            # V-term readback
            vrec_lo = cp.tile([27, 1024], bf16, tag="vrec0")
            vrec_hi = cp.tile([27, 1024], bf16, tag="vrec1")
            sync.dma_start(vrec_lo, ag_out[2][bass.ds(sv_lo, 1), :, :])
            sync.dma_start(vrec_hi, ag_out[2][bass.ds(sv_hi, 1), :, :])

            # ---------------- flow head ----------------
            yp = xp.tile([27, PD, PL, PL], bf16, tag="xslot1")
            nc.gpsimd.memset(yp, 0.0)
            flow_v = flowd[:, :].rearrange("p (a b c) -> p a b c", a=PD, b=32, c=32)

            def s2_plane(d):
                for h in range(2):
                    pt2 = ps.tile([3, 16, 32], f32, tag="psacc", name=f"fs2_{d}{h}")
                    first = True
                    for j in range(9):
                        ky, kx = j // 3, j % 3
                        rhs = yp[0:27, d, 16 * h + ky: 16 * h + ky + 16, kx: kx + 32]
                        nc.tensor.matmul(pt2, e_sb[:, 3 * j: 3 * j + 3], rhs,
                                         start=first, stop=(j == 8))
                        first = False
                    fo = scr.tile([3, 16, 32], f32, tag=f"fscr{h % 2}", name=f"fo{d}{h}")
                    act.activation(fo, pt2, AF.Identity, bias=bo_sb, scale=1.0)
                    sync.dma_start(flow_v[:, d, 16 * h: 16 * h + 16, :], fo)

            # stage-1 interior pass (planes 1, 2), weight-stationary
            pts1 = [ps.tile([27, 16, 32], f32, tag="psacc", name=f"fs1i{j}")
                    for j in range(4)]
            for kz in range(3):
                for k in range(2):
                    for j, (d, h) in enumerate(INT_BLOCKS):
                        rhs = cn[k][:, d + kz, 16 * h + 1: 16 * h + 17, 1:33]
                        nc.tensor.matmul(pts1[j], wo_sb[k][:, kz, :], rhs,
                                         start=(kz == 0 and k == 0),
                                         stop=(kz == 2 and k == 1))
            for j, (d, h) in enumerate(INT_BLOCKS):
                act.activation(yp[:, d, 16 * h + 1: 16 * h + 17, 1:33], pts1[j], AF.Copy)
            for d in (1, 2):
                s2_plane(d)

            # stage-1 boundary pass: own-kz terms + received V terms
            bnd_kzs = {0: (1, 2), 3: (0, 1)}
            pts1b = [ps.tile([27, 16, 32], f32, tag="psacc", name=f"fs1b{j}")
                     for j in range(4)]
            for j, (d, h) in enumerate(BND_BLOCKS):
                kzs = bnd_kzs[d]
                first = True
                for kz in kzs:
                    for k in range(2):
                        rhs = cn[k][:, d + kz, 16 * h + 1: 16 * h + 17, 1:33]
                        nc.tensor.matmul(pts1b[j], wo_sb[k][:, kz, :], rhs,
                                         start=first,
                                         stop=(kz == kzs[-1] and k == 1))
                        first = False
            for j, (d, h) in enumerate(BND_BLOCKS):
                act.activation(yp[:, d, 16 * h + 1: 16 * h + 17, 1:33], pts1b[j], AF.Copy)
            for d, vrec, mcol in ((0, vrec_lo, 0), (3, vrec_hi, 1)):
                for h in range(2):
                    vv = vrec[:, 512 * h: 512 * h + 512].rearrange(
                        "p (a b) -> p a b", a=16, b=32)
                    dst = yp[0:27, d, 16 * h + 1: 16 * h + 17, 1:33]
                    vec.scalar_tensor_tensor(dst, vv, mask_sb[0:27, mcol: mcol + 1],
                                             dst, ALU.mult, ALU.add)
            for d in (0, 3):
                s2_plane(d)

    nc.compile()
    return nc


# ---------------------------------------------------------------------------
# host-side input preparation
# ---------------------------------------------------------------------------

def host_prep(src, tgt, C, up_w, up_b, c1_w, c1_b, c2_w, c2_b, out_w, out_b):
    """Build the 8 per-core in_maps (numpy, bf16 compute precision)."""
    src = np.asarray(src); tgt = np.asarray(tgt); C = np.asarray(C)
    rows = cost_row_map()

    T1 = np.ascontiguousarray(np.transpose(np.asarray(c1_w), (1, 2, 3, 4, 0))).reshape(CFULL, 27, CFULL)
    w1k0 = T1[0:128].astype(BF16)
    w1k1 = np.zeros((128, 27, CFULL), np.float32)
    w1k1[0:64] = T1[155:219]                      # Cup channels at rows 0-63
    for t, r in rows.items():
        w1k1[r] = T1[128 + t]                     # cost channels at rows 64-90
    w1k1 = w1k1.astype(BF16)

    T2 = np.ascontiguousarray(np.transpose(np.asarray(c2_w), (1, 2, 3, 4, 0))).reshape(CFULL, 27, CFULL)
    w2k0 = T2[0:128].astype(BF16)
    w2k1 = np.zeros((128, 27, CFULL), np.float32)
    w2k1[0:CC1] = T2[128:219]
    w2k1 = w2k1.astype(BF16)

    # upconv: wc[o, ci, k] = up_w[ci, o, 3-k]; Wu[ci, p, t, o] = wc[o, ci, 2t+p]
    wc = np.flip(np.transpose(np.asarray(up_w), (1, 0, 2, 3, 4)), axis=(2, 3, 4))
    wcr = np.transpose(wc, (1, 2, 3, 4, 0))       # [ci, kz, ky, kx, o]
    Wu = np.zeros((512, 8, 8, 64), np.float32)
    for p in range(8):
        pz, py, px = p >> 2 & 1, p >> 1 & 1, p & 1
        for t in range(8):
            tz, ty, tx = t >> 2 & 1, t >> 1 & 1, t & 1
            Wu[0:411, p, t, :] = wcr[:, 2 * tz + pz, 2 * ty + py, 2 * tx + px, :]
    wu = [Wu[128 * k: 128 * k + 128].reshape(128, 4096).astype(BF16) for k in range(4)]

    To = np.transpose(np.asarray(out_w), (1, 2, 3, 4, 0)).reshape(CFULL, 3, 27)
    wo0 = To[0:128].reshape(128, 81).astype(BF16)
    wo1 = np.zeros((128, 3, 27), np.float32)
    wo1[0:CC1] = To[128:219]
    wo1 = wo1.reshape(128, 81).astype(BF16)
    ec = np.zeros((27, 27), np.float32)
    for j in range(9):
        for o in range(3):
            ec[3 * j + o, 3 * j + o] = 1.0
    ec = ec.astype(BF16)

    red = np.zeros((128, 32), np.float32)
    red[0:64, 0] = 1.0 / 64
    red[64:128, 1] = 1.0 / 64
    red = red.astype(BF16)

    c1_b = np.asarray(c1_b); c2_b = np.asarray(c2_b)
    b1c0 = c1_b[0:128].reshape(128, 1).astype(np.float32)
    b1c1 = np.zeros((128, 1), np.float32); b1c1[0:CC1, 0] = c1_b[128:219]
    b2c0 = c2_b[0:128].reshape(128, 1).astype(np.float32)
    b2c1 = np.zeros((128, 1), np.float32); b2c1[0:CC1, 0] = c2_b[128:219]
    bu = np.asarray(up_b).reshape(64, 1).astype(np.float32)
    bo = np.asarray(out_b).reshape(3, 1).astype(np.float32)

    shared = {
        "wu0": wu[0], "wu1": wu[1], "wu2": wu[2], "wu3": wu[3],
        "w1k0": w1k0.reshape(128, 27 * CFULL), "w1k1": w1k1.reshape(128, 27 * CFULL),
        "w2k0": w2k0.reshape(128, 27 * CFULL), "w2k1": w2k1.reshape(128, 27 * CFULL),
        "wo0": wo0, "wo1": wo1, "ec": ec, "red": red,
        "b1c0": b1c0, "b1c1": b1c1, "b2c0": b2c0, "b2c1": b2c1,
        "bu": bu, "bo": bo,
    }

    ts = np.concatenate([tgt[0], src[0]], axis=0).astype(BF16)  # [128, 32, 32, 32]
    Cb = C[0].astype(BF16)                                      # [411, 16, 16, 16]

    in_maps = []
    for k in range(NCORES):
        x0 = np.zeros((128, NPLANES, PL, PL), BF16)
        zlo = 4 * k - 1
        for zp in range(NPLANES):
            z = zlo + zp
            if 0 <= z < D:
                x0[:, zp, 1:33, 1:33] = ts[:, z]
        cs = []
        for kc in range(4):
            a = np.zeros((128, 4, 18, 18), BF16)
            for ip in range(4):
                z = 2 * k - 1 + ip
                if 0 <= z < 16:
                    c0 = 128 * kc
                    n = min(128, 411 - c0)
                    if n > 0:
                        a[0:n, ip, 1:17, 1:17] = Cb[c0:c0 + n, z]
            cs.append(a.reshape(128, 4 * CP))
        mask = np.zeros((128, 2), np.float32)
        mask[:, 0] = 1.0 if k > 0 else 0.0
        mask[:, 1] = 1.0 if k < 7 else 0.0
        hidx = np.array([[2 * (k - 1) + 1 if k > 0 else 0,
                          2 * (k + 1) if k < 7 else 0]], np.uint32)
        m = dict(shared)
        m["x0"] = x0.reshape(128, NPLANES * PLSZ)
        for kc in range(4):
            m[f"cs{kc}"] = cs[kc]
        m["mask"] = mask
        m["hidx"] = hidx
        in_maps.append(m)
    return in_maps


def assemble_outputs(results):
    Cn = np.zeros((219, 32, 32, 32), np.float32)
    out = np.zeros((3, 32, 32, 32), np.float32)
    for k, r in enumerate(results):
        cnb = np.concatenate([np.asarray(r["cn0"]).astype(np.float32),
                              np.asarray(r["cn1"]).astype(np.float32)], axis=0)
        cnb = cnb.reshape(CFULL, PD, PL, PL)[:, :, 1:33, 1:33]
        Cn[:, 4 * k: 4 * k + 4] = cnb
        out[:, 4 * k: 4 * k + 4] = np.asarray(r["flow"]).reshape(3, PD, 32, 32)
    return Cn[None], out[None]


_PROG = None


def _get_prog():
    global _PROG
    if _PROG is None:
        _PROG = build_program()
    return _PROG


def kernel(**inputs):
    nc = _get_prog()
    in_maps = host_prep(**inputs)
    from concourse.bass_utils import run_bass_kernel_spmd
    res = run_bass_kernel_spmd(nc, in_maps, core_ids=list(range(NCORES)))
    return assemble_outputs(res.results)
